# revision 1
# baseline (speedup 1.0000x reference)
"""ContrastiveLoss kernel for 8 Trainium2 NeuronCores (Bass/Tile).

Strategy (sharding hint): shard z by rows across 8 cores. Each core
normalizes + transposes its [1024, 1024] slab (PE transpose), casts to
fp8e4, AllGathers the normalized-transposed slabs (1MB -> 8.4MB), then
computes its [1024, 8192] slab of the cosine-similarity matrix with
fp8 DoubleRow matmuls (2 k-planes per call, 0.5 cycles/row), doing a
fused exp(x/T) + row-sum on the scalar engine (no max-subtraction
needed: logits are bounded by 1/T). The diagonal self-term is removed
by recomputing the local self-block with bit-identical fp8 matmuls and
subtracting its exp. Positives sim[i, (i-4096)%8192] are computed in
fp32 as row-wise dots of z_local with the positive slab (a host-sliced
input), so the instruction stream is identical on every core - only
data differs. Output: per-row NLL [128, 8] per core; host gathers and
takes the mean.
"""
import numpy as np

import concourse.bacc as bacc
from concourse import mybir
from concourse.tile import TileContext
from concourse.bass_utils import run_bass_kernel_spmd

N, D, C = 8192, 1024, 8
L = N // C            # rows per core
P = 128               # partitions
MT = L // P           # 8 row-tiles per core
KT = D // P           # 8 contraction chunks
KK = KT // 2          # 4 DoubleRow pair chunks
NB = 512              # matmul moving-dim tile
CB = N // NB          # 16 column blocks
TEMP = 0.07
SCALE = 1.0 / TEMP
EPS = 1e-8

F32 = mybir.dt.float32
DT = mybir.dt.float8e4  # matmul operand dtype (DoubleRow perf mode)
DR = mybir.MatmulPerfMode.DoubleRow

AF = mybir.ActivationFunctionType
ALU = mybir.AluOpType

_cached = {}


def _emit_pipeline(nc, z, zp, imf, lhs2, rnz, rnp, posd, Stiles, nll_sb,
                   ag_in, ag_out, zpool, rpool, epool, spool, pbig, psmall,
                   phases="ABCD"):
    # ---------------- Phase A: normalize + transpose local slab.
    # z-only chain first: it gates the AllGathers; zp/positives work is
    # emitted later (phase A2) so it runs inside the collective window.
    for m in range(MT):
        zt = zpool.tile([P, D], F32, tag="zt", name="zt")
        nc.sync.dma_start(out=zt[:, :], in_=z[m * P:(m + 1) * P, :])
        ssq = spool.tile([P, 1], F32, tag="ssq", name="ssq")
        scr = zpool.tile([P, D], F32, tag="scr", name="scr")
        nc.scalar.activation(scr[:, :], zt[:, :], AF.Square,
                             accum_out=ssq[:, 0:1])
        nrm = spool.tile([P, 1], F32, tag="nrm", name="nrm")
        nc.scalar.activation(nrm[:, 0:1], ssq[:, 0:1], AF.Sqrt)
        nc.vector.tensor_scalar_max(nrm[:, 0:1], nrm[:, 0:1], EPS)
        nc.vector.reciprocal(rnz[:, m:m + 1], nrm[:, 0:1])
        zn = zpool.tile([P, D], F32, tag="zn", name="zn")
        nc.scalar.activation(zn[:, :], zt[:, :], AF.Copy,
                             scale=rnz[:, m:m + 1])
        for k in range(KT):
            pt = psmall.tile([P, P], F32, tag="small", name="pt")
            nc.tensor.transpose(pt[:, :], zn[:, k * P:(k + 1) * P], imf[:, :])
            nc.vector.tensor_copy(
                lhs2[k // 2][:, k % 2, m * P:(m + 1) * P], pt[:, :])

    # ---------------- Phase B: chunked AllGather fp8 znT (overlaps C)
    H = L // 2
    if "B" in phases:
        for h in range(2):
            for kk in range(KK):
                nc.sync.dma_start(
                    out=ag_in[h][kk, :, :, :],
                    in_=lhs2[kk][:, :, h * H:(h + 1) * H])
            nc.gpsimd.collective_compute(
                "AllGather", ALU.bypass,
                ins=[ag_in[h].ap().opt()],
                outs=[ag_out[h].ap().opt()],
                replica_groups=[list(range(C))],
            )

    # ---------------- Phase A2: positives (overlaps the AllGathers)
    for m in range(MT):
        zt = zpool.tile([P, D], F32, tag="zt", name="zt2")
        nc.sync.dma_start(out=zt[:, :], in_=z[m * P:(m + 1) * P, :])
        zpt = zpool.tile([P, D], F32, tag="zpt", name="zpt")
        nc.sync.dma_start(out=zpt[:, :], in_=zp[m * P:(m + 1) * P, :])
        ssq2 = spool.tile([P, 1], F32, tag="ssq2", name="ssq2")
        scr2 = zpool.tile([P, D], F32, tag="scr2", name="scr2")
        nc.scalar.activation(scr2[:, :], zpt[:, :], AF.Square,
                             accum_out=ssq2[:, 0:1])
        nrm2 = spool.tile([P, 1], F32, tag="nrm2", name="nrm2")
        nc.scalar.activation(nrm2[:, 0:1], ssq2[:, 0:1], AF.Sqrt)
        nc.vector.tensor_scalar_max(nrm2[:, 0:1], nrm2[:, 0:1], EPS)
        nc.vector.reciprocal(rnp[:, m:m + 1], nrm2[:, 0:1])
        scr3 = zpool.tile([P, D], F32, tag="scr3", name="scr3")
        nc.vector.tensor_mul(scr3[:, :], zt[:, :], zpt[:, :])
        nc.vector.reduce_sum(posd[:, m:m + 1], scr3[:, :],
                             axis=mybir.AxisListType.X)

    # ---------------- Phase C: similarity slab + exp row-sums
    if "C" not in phases:
        return
    for cb in range(CB):
        h, r = cb // C, cb % C
        rts = []
        for kk in range(KK):
            rt = rpool.tile([P, 2, NB], DT, tag=f"rhs{kk}", name=f"rt{kk}")
            nc.sync.dma_start(
                out=rt[:, :, :],
                in_=ag_out[h][r, kk, :, :, :])
            rts.append(rt)
        for m in range(MT):
            ps = pbig.tile([P, NB], F32, tag="big", name="ps")
            for kk in range(KK):
                nc.tensor.matmul(ps[:, :],
                                 lhs2[kk][:, :, m * P:(m + 1) * P],
                                 rts[kk][:, :, :],
                                 start=(kk == 0), stop=(kk == KK - 1),
                                 perf_mode=DR)
            esc = epool.tile([P, NB], F32, tag="esc", name="esc")
            nc.scalar.activation(esc[:, :], ps[:, :], AF.Exp, scale=SCALE,
                                 accum_out=Stiles[m][:, cb:cb + 1])

    # ---------------- Phase D: self-term removal + NLL
    if "D" not in phases:
        return
    for m in range(MT):
        pss = psmall.tile([P, P], F32, tag="small", name="pss")
        for kk in range(KK):
            nc.tensor.matmul(pss[:, :],
                             lhs2[kk][:, :, m * P:(m + 1) * P],
                             lhs2[kk][:, :, m * P:(m + 1) * P],
                             start=(kk == 0), stop=(kk == KK - 1),
                             perf_mode=DR)
        dscr = epool.tile([P, P], F32, tag="dscr", name="dscr")
        dv = spool.tile([P, 1], F32, tag="dv", name="dv")
        nc.vector.tensor_mul(dscr[:, :], pss[:, :], imf[:, :])
        nc.vector.reduce_sum(dv[:, 0:1], dscr[:, :],
                             axis=mybir.AxisListType.X)
        es = spool.tile([P, 1], F32, tag="es", name="es")
        nc.scalar.activation(es[:, 0:1], dv[:, 0:1], AF.Exp, scale=SCALE)
        sr = spool.tile([P, 1], F32, tag="sr", name="sr")
        nc.vector.reduce_sum(sr[:, 0:1], Stiles[m][:, :],
                             axis=mybir.AxisListType.X)
        sc = spool.tile([P, 1], F32, tag="sc", name="sc")
        nc.vector.tensor_sub(sc[:, 0:1], sr[:, 0:1], es[:, 0:1])
        lse = spool.tile([P, 1], F32, tag="lse", name="lse")
        nc.scalar.activation(lse[:, 0:1], sc[:, 0:1], AF.Ln)
        pr = spool.tile([P, 1], F32, tag="pr", name="pr")
        nc.vector.scalar_tensor_tensor(
            out=pr[:, 0:1], in0=posd[:, m:m + 1], scalar=rnz[:, m:m + 1],
            in1=rnp[:, m:m + 1], op0=ALU.mult, op1=ALU.mult)
        nc.vector.scalar_tensor_tensor(
            out=nll_sb[:, m:m + 1], in0=pr[:, 0:1], scalar=-SCALE,
            in1=lse[:, 0:1], op0=ALU.mult, op1=ALU.add)


def _build(reps: int = 1, phases: str = "ABCD"):
    nc = bacc.Bacc(trn_type="TRN2")
    z = nc.dram_tensor("z", [L, D], F32, kind="ExternalInput")
    zp = nc.dram_tensor("zp", [L, D], F32, kind="ExternalInput")
    im = nc.dram_tensor("im", [P, P], F32, kind="ExternalInput")
    nll_out = nc.dram_tensor("nll", [P, MT], F32, kind="ExternalOutput")

    H = L // 2
    ag_in = [nc.dram_tensor(f"ag_in{h}", [KK, P, 2, H], DT)
             for h in range(2)]
    ag_out = [nc.dram_tensor(f"ag_out{h}", [C, KK, P, 2, H], DT,
                             addr_space="Shared") for h in range(2)]

    with TileContext(nc) as tc:
        with (
            tc.tile_pool(name="const", bufs=1) as cpool,
            tc.tile_pool(name="lhs", bufs=1) as lpool,
            tc.tile_pool(name="stat", bufs=1) as spool,
            tc.tile_pool(name="prep", bufs=2) as zpool,
            tc.tile_pool(name="rhs", bufs=3) as rpool,
            tc.tile_pool(name="esc", bufs=3) as epool,
            tc.tile_pool(name="pbig", bufs=6, space="PSUM") as pbig,
            tc.tile_pool(name="psmall", bufs=2, space="PSUM") as psmall,
        ):
            imf = cpool.tile([P, P], F32, tag="imf")
            nc.sync.dma_start(out=imf[:, :], in_=im[:, :])

            lhs2 = [lpool.tile([P, 2, L], DT, tag=f"lhs{kk}", name=f"lhs{kk}")
                    for kk in range(KK)]
            rnz = spool.tile([P, MT], F32, tag="rnz")
            rnp = spool.tile([P, MT], F32, tag="rnp")
            posd = spool.tile([P, MT], F32, tag="posd")
            Stiles = [spool.tile([P, CB], F32, tag=f"S{m}", name=f"S{m}")
                      for m in range(MT)]
            nll_sb = spool.tile([P, MT], F32, tag="nll")
            nc.vector.memset(nll_sb[:, :], 0.0)

            for _rep in range(reps):
                _emit_pipeline(nc, z, zp, imf, lhs2, rnz, rnp, posd,
                               Stiles, nll_sb, ag_in, ag_out,
                               zpool, rpool, epool, spool, pbig, psmall,
                               phases=phases)

            nc.sync.dma_start(out=nll_out[:, :], in_=nll_sb[:, :])

    nc.finalize()
    return nc


def _build_repeat(reps: int, phases: str = "ABCD"):
    return _build(reps, phases)


def get_nc():
    if "nc" not in _cached:
        _cached["nc"] = _build()
    return _cached["nc"]


def kernel(z: np.ndarray, _profile: dict | None = None) -> np.ndarray:
    assert z.shape == (N, D)
    z = np.ascontiguousarray(z, dtype=np.float32)
    imask = np.eye(P, dtype=np.float32)
    in_maps = []
    for c in range(C):
        cp = (c + 4) % C
        in_maps.append({
            "z": z[c * L:(c + 1) * L],
            "zp": z[cp * L:(cp + 1) * L],
            "im": imask,
        })
    nc = get_nc()
    res = run_bass_kernel_spmd(nc, in_maps, core_ids=list(range(C)))
    if _profile is not None:
        _profile["exec_time_ns"] = res.exec_time_ns
        _profile["results"] = res
    # nll layout per core: [p, m] -> global row c*L + m*P + p
    total = 0.0
    for c in range(C):
        total += float(res.results[c]["nll"].sum(dtype=np.float64))
    return np.float32(total / N)



# revision 51
# speedup vs baseline: 1.4916x; 1.4916x over previous
"""ContrastiveLoss kernel for 8 Trainium2 NeuronCores (Bass/Tile).

Strategy: shard z by rows across 8 cores. Each core normalizes +
transposes its [1024, 1024] slab (PE transpose), casts to fp8e4 into
lhs_all, and stages it to private DRAM. The all-gather of the 8MB fp8
payload is PARITY-SPLIT: one AllGather with replica groups
[[0,2,4,6],[1,3,5,7]] in which even cores gather the 4 even slabs and
odd cores the 4 odd slabs (4MB output each, half the collective wire
time), writing into pair-shared HBM (addr_space="Shared") at a
parity-selected offset (register-driven AP). Each core then sees all 8
slabs: its own parity half from its own collective, the other half
written by its HBM pair partner; a tiny 8-core AllReduce barrier
orders partner-half reads.

Each core computes its [1024, 8192] slab of the cosine-similarity
matrix with fp8 DoubleRow matmuls into [128, 2048] PSUM tiles (4 banks)
and one fused exp(x/T)+row-sum accumulation per tile on the scalar
engine (no max-subtraction needed: logits bounded by 1/T). The
diagonal self-term is removed by recomputing the local self-block with
numerically identical fp8 matmuls and subtracting its exp. Positives
sim[i, (i-4096)%8192] are computed in fp32 as row-wise dots of z_local
with the positive slab (a host-sliced input). Activation-table loads
are minimized to 2 (Square/Rsqrt table, then Exp/Ln table). Output:
per-row NLL [128, 8] per core; host gathers and takes the mean.
"""
import numpy as np

import concourse.bacc as bacc
import concourse.bass as bass
from concourse import mybir
from concourse.tile import TileContext
from concourse.tile_rust import add_dep_helper
from concourse.bass_utils import run_bass_kernel_spmd

N, D, C = 8192, 1024, 8
L = N // C            # rows per core
P = 128               # partitions
MT = L // P           # 8 row-tiles per core
KT = D // P           # 8 contraction chunks
KK = KT // 2          # 4 DoubleRow pair chunks
C2 = C // 2           # slabs per parity half
TQ = 4                # column quarters (each 2048 cols = 2 slabs)
TQ1 = TQ + 1          # +1 accum slot for the split first tile
TEMP = 0.07
SCALE = 1.0 / TEMP
EPS = 1e-8

F32 = mybir.dt.float32
I32 = mybir.dt.int32
DT = mybir.dt.float8e4  # matmul operand dtype (DoubleRow perf mode)
DR = mybir.MatmulPerfMode.DoubleRow

AF = mybir.ActivationFunctionType
ALU = mybir.AluOpType
AX = mybir.AxisListType

_cached = {}


def _emit_pipeline(nc, tc, z, zp, im, par, pnot, ag_in, gpriv, gath, bar,
                   nll_out, pools, phases="ABCD"):
    (cpool, zres, lpool, spool, wpool, rts, epool, ps) = pools

    imf = cpool.tile([P, P], F32, tag="imf")
    nc.sync.dma_start(out=imf[:, :], in_=im[:, :])
    par_sb = cpool.tile([1, 1], I32, tag="par")
    nc.sync.dma_start(out=par_sb[:, :], in_=par[:, :])
    pnot_sb = cpool.tile([1, 1], I32, tag="pnot")
    nc.sync.dma_start(out=pnot_sb[:, :], in_=pnot[:, :])

    lhs_all = lpool.tile([P, KK, 2, L], DT, tag="lhs")
    ssq = spool.tile([P, MT], F32, tag="ssq")
    ssqc = spool.tile([P, MT], F32, tag="ssqc")
    rn = spool.tile([P, MT], F32, tag="rn")
    S = spool.tile([P, MT, TQ1], F32, tag="S")
    nc.vector.memset(S[:, :, TQ:TQ1], 0.0)
    dv = spool.tile([P, MT], F32, tag="dv")
    sp = spool.tile([P, MT], F32, tag="sp")
    spc = spool.tile([P, MT], F32, tag="spc")
    rnp = spool.tile([P, MT], F32, tag="rnp")
    posd = spool.tile([P, MT], F32, tag="posd")
    srow = spool.tile([P, MT], F32, tag="srow")
    es = spool.tile([P, MT], F32, tag="es")
    sc = spool.tile([P, MT], F32, tag="sc")
    lse = spool.tile([P, MT], F32, tag="lse")
    pr = spool.tile([P, MT], F32, tag="pr")
    pr2 = spool.tile([P, MT], F32, tag="pr2")
    nll_sb = spool.tile([P, MT], F32, tag="nll")

    zts = []

    # ---------------- Phase A: normalize + transpose local slab -> fp8.
    # Fully per-m pipelined: each m streams load -> square -> sqrt ->
    # recip -> scale -> transpose x8 -> fp8 copy -> stage, so the chain
    # runs at z-load pace instead of barriering on all 8 norms.
    nrm = spool.tile([P, MT], F32, tag="nrm")
    for m in range(MT):
        zt = zres.tile([P, D], F32, tag=f"zt{m}", name=f"zt{m}")
        nc.sync.dma_start(out=zt[:, :], in_=z[m * P:(m + 1) * P, :])
        zts.append(zt)
        scr = wpool.tile([P, D], F32, tag="scr", name="scr")
        nc.scalar.activation(scr[:, :], zt[:, :], AF.Square,
                             accum_out=ssq[:, m:m + 1])
        nc.vector.tensor_scalar_max(ssqc[:, m:m + 1], ssq[:, m:m + 1],
                                    EPS * EPS)
        nc.scalar.activation(nrm[:, m:m + 1], ssqc[:, m:m + 1], AF.Sqrt)
        nc.vector.reciprocal(rn[:, m:m + 1], nrm[:, m:m + 1])
        zn = wpool.tile([P, D], F32, tag="zn", name="zn")
        if m % 2 == 0:
            nc.scalar.activation(zn[:, :], zt[:, :], AF.Copy,
                                 scale=rn[:, m:m + 1])
        else:
            # alternate the scale between ACT and DVE so phase A streams
            # at z-load pace instead of serializing on one engine
            nc.vector.tensor_scalar(zn[:, :], zt[:, :], rn[:, m:m + 1],
                                    None, ALU.mult)
        # all 8 k-chunk transposes into one 2-bank psum tile, then a
        # single [128, 1024] fp8-cast copy (fewer PE<->DVE handoffs)
        pt = ps.tile([P, D], F32, tag="big", name="pt")
        for k in range(KT):
            nc.tensor.transpose(pt[:, k * P:(k + 1) * P],
                                zn[:, k * P:(k + 1) * P], imf[:, :])
        nc.vector.tensor_copy(
            lhs_all[:, :, :, m * P:(m + 1) * P], pt[:, :])
        # stage this m-chunk of the fp8 slab out for the collective
        si = nc.sync.dma_start(out=ag_in[:, :, :, m * P:(m + 1) * P],
                               in_=lhs_all[:, :, :, m * P:(m + 1) * P])
    last_stage = si

    # ---------------- Phase B: parity-split AllGather + share + barrier.
    # My own parity half goes gpriv -> SBUF rt tiles (feeding phase C
    # immediately), and each rt tile is re-published SBUF -> pair-shared
    # HBM for my partner (SBUF-path DMAs are 2x cheaper in the model
    # than DRAM->DRAM). A 1-byte 8-core AllGather is the barrier
    # (AllReduce pays 1.875x on the 15us constant; AllGather doesn't).
    gi = None
    bi = []
    rt_tiles = {}
    if "B" in phases:
        gi = nc.gpsimd.collective_compute(
            "AllGather", ALU.bypass,
            ins=[ag_in.ap().opt()],
            outs=[gpriv.ap().opt()],
            replica_groups=[[0, 2, 4, 6], [1, 3, 5, 7]],
        )
        pv_reg = nc.sync.alloc_register("pv")
        nc.sync.reg_load(pv_reg, par_sb[0:1, 0:1])
        pv = nc.sync.snap(pv_reg)
        # own-half loads + publishes interleaved on SP (HWDGE frees the
        # sequencer during transfers; ACT must keep the exp stream and
        # Pool SWDGE would serialize with the barrier collectives):
        # rt00 rt01 pub0 pub1 rt02 rt03 pub2 pub3, so barrier 1 fires
        # while own-half exps still stream
        pubs = []
        for idx in range(C2):
            rt = rts.tile([P, KK, 2, L], DT, tag=f"rt0{idx}",
                          name=f"rt0{idx}")
            nc.sync.dma_start(out=rt[:, :, :, :],
                              in_=gpriv.ap()[idx, :, :, :, :])
            rt_tiles[(0, idx)] = rt
            if idx % 2 == 1:
                for j in (idx - 1, idx):
                    pubs.append(nc.sync.dma_start(
                        out=gath.ap()[bass.ds(pv, 1), j].squeeze(0),
                        in_=rt_tiles[(0, j)][:, :, :, :]))
        # two staggered barriers: the first fires once slabs 0-1 are
        # published so the partner can start its hp=1 work ~15us earlier
        bi = []
        for h in range(2):
            b = nc.gpsimd.collective_compute(
                "AllGather", ALU.bypass,
                ins=[ag_in.ap()[0:1, 0:1, 0:1, 0:1].opt()],
                outs=[bar.ap()[h].unsqueeze(0).opt()],
                replica_groups=[list(range(C))],
            )
            for pb in pubs[2 * h:2 * h + 2]:
                add_dep_helper(b.ins, pb.ins, reason="barrier after publish")
            bi.append(b)

    # ---------------- Phase A2: positives (overlaps the gather window).
    for m in range(MT):
        zpt = wpool.tile([P, D], F32, tag="zpt", name="zpt")
        # zp loads on the ACT HWDGE queue: they execute in-order with the
        # A2 squares during the gather window (a straggler on SP gets
        # statically scheduled mid-exp-stream and thrashes the act table)
        di = nc.scalar.dma_start(out=zpt[:, :], in_=zp[m * P:(m + 1) * P, :])
        add_dep_helper(di.ins, last_stage.ins, reason="zp after staging")
        scr2 = wpool.tile([P, D], F32, tag="scr2", name="scr2")
        nc.scalar.activation(scr2[:, :], zpt[:, :], AF.Square,
                             accum_out=sp[:, m:m + 1])
        pd = wpool.tile([P, D], F32, tag="pd", name="pd")
        nc.vector.tensor_mul(pd[:, :], zts[m][:, :], zpt[:, :])
        nc.vector.reduce_sum(posd[:, m:m + 1], pd[:, :], axis=AX.X)
    nc.vector.tensor_scalar_max(spc[:, :], sp[:, :], EPS * EPS)
    nrmp = spool.tile([P, MT], F32, tag="nrmp")
    sqi = nc.scalar.activation(nrmp[:, :], spc[:, :], AF.Sqrt)
    nc.vector.reciprocal(rnp[:, :], nrmp[:, :])
    # pin the Exp+Ln activation table now (inside the gather window) so
    # no table reload lands on the critical exp stream or final Ln
    lnpin = spool.tile([P, 1], F32, tag="lnpin")
    lni = nc.scalar.activation(lnpin[:, 0:1], spc[:, 0:1], AF.Ln)
    # make the pin the LAST sqrt->ln-table transition on ACT so every
    # later activation (exps + final Ln) runs from one table
    add_dep_helper(lni.ins, sqi.ins, reason="ln pin after last sqrt")

    # ---------------- Phase D1: self-block diagonal (gather window).
    # Numerically identical fp8 matmuls to the slab-my-cid block of
    # phase C, so subtracting exp(dv/T) removes the self term exactly.
    for m in range(MT):
        pss = ps.tile([P, P], F32, tag="big", name="pss")
        for kk in range(KK):
            nc.tensor.matmul(pss[:, :],
                             lhs_all[:, kk, :, m * P:(m + 1) * P],
                             lhs_all[:, kk, :, m * P:(m + 1) * P],
                             start=(kk == 0), stop=(kk == KK - 1),
                             perf_mode=DR)
        dscr = epool.tile([P, P], F32, tag="dscr", name="dscr")
        nc.vector.tensor_mul(dscr[:, :], pss[:, :], imf[:, :])
        nc.vector.reduce_sum(dv[:, m:m + 1], dscr[:, :], axis=AX.X)
    # self-term exp, off the critical tail (runs in the gather window)
    esi = nc.scalar.activation(es[:, :], dv[:, :], AF.Exp, scale=SCALE)
    add_dep_helper(esi.ins, lni.ins, reason="exp after ln-table pin")

    # ---------------- Phase C: similarity slab + fused exp row-sums.
    if "C" not in phases:
        return
    # partner half from pair-shared HBM, gated on the barrier
    pn_reg = nc.sync.alloc_register("pn")
    nc.sync.reg_load(pn_reg, pnot_sb[0:1, 0:1])
    pn = nc.sync.snap(pn_reg)
    for idx in range(C2):
        rt = rts.tile([P, KK, 2, L], DT, tag=f"rt1{idx}", name=f"rt1{idx}")
        src = gath.ap()[bass.ds(pn, 1), idx].squeeze(0)
        di = nc.sync.dma_start(out=rt[:, :, :, :], in_=src)
        if bi:
            add_dep_helper(di.ins, bi[idx // 2].ins,
                           reason="partner half after barrier")
        rt_tiles[(1, idx)] = rt
    # own-parity half (hp=0, from gpriv) first, partner half after
    for t in (0, 1, 2, 3):
        hp, i0 = t // 2, (t % 2) * 2
        for m in range(MT):
            pt2 = ps.tile([P, 4 * 512], F32, tag="big", name="pt2")
            for s in range(4):
                rt = rt_tiles[(hp, i0 + s // 2)]
                cs = (s % 2) * 512
                for kk in range(KK):
                    nc.tensor.matmul(pt2[:, s * 512:(s + 1) * 512],
                                     lhs_all[:, kk, :, m * P:(m + 1) * P],
                                     rt[:, kk, :, cs:cs + 512],
                                     start=(kk == 0), stop=(kk == KK - 1),
                                     perf_mode=DR)
            esc = epool.tile([P, 4 * 512], F32, tag="esc", name="esc")
            tileno = t * MT + m
            if tileno < 22:
                # early tiles: plain exp on ACT (no 187ns accum-read),
                # row-sum on the otherwise-idle DVE
                ei = nc.scalar.activation(esc[:, :], pt2[:, :], AF.Exp,
                                          scale=SCALE)
                nc.vector.reduce_sum(S[:, m, t:t + 1].opt(), esc[:, :],
                                     axis=AX.X)
            else:
                # late tiles keep the fused accum so nothing trails the
                # final exp
                ei = nc.scalar.activation(esc[:, :], pt2[:, :], AF.Exp,
                                          scale=SCALE,
                                          accum_out=S[:, m, t:t + 1].opt())
            if t == 0 and m == 0:
                # pin the whole exp stream after the ln-table pin so the
                # static schedule keeps one activation table throughout
                add_dep_helper(ei.ins, esi.ins, reason="exps after ln pin")

    # ---------------- Phase D2: NLL finale.
    if "D" not in phases:
        return
    nc.vector.reduce_sum(srow[:, :].unsqueeze(2), S[:, :, :], axis=AX.X)
    nc.vector.tensor_sub(sc[:, :], srow[:, :], es[:, :])
    nc.scalar.activation(lse[:, :], sc[:, :], AF.Ln)
    nc.vector.tensor_mul(pr[:, :], posd[:, :], rn[:, :])
    nc.vector.tensor_mul(pr2[:, :], pr[:, :], rnp[:, :])
    nc.vector.scalar_tensor_tensor(
        out=nll_sb[:, :], in0=pr2[:, :], scalar=-SCALE,
        in1=lse[:, :], op0=ALU.mult, op1=ALU.add)
    nc.sync.dma_start(out=nll_out[:, :], in_=nll_sb[:, :])


def _build(reps: int = 1, phases: str = "ABCD"):
    nc = bacc.Bacc(trn_type="TRN2")
    z = nc.dram_tensor("z", [L, D], F32, kind="ExternalInput")
    zp = nc.dram_tensor("zp", [L, D], F32, kind="ExternalInput")
    im = nc.dram_tensor("im", [P, P], F32, kind="ExternalInput")
    par = nc.dram_tensor("par", [1, 1], I32, kind="ExternalInput")
    pnot = nc.dram_tensor("pnot", [1, 1], I32, kind="ExternalInput")
    nll_out = nc.dram_tensor("nll", [P, MT], F32, kind="ExternalOutput")

    ag_in = nc.dram_tensor("ag_in", [P, KK, 2, L], DT)
    gpriv = nc.dram_tensor("gpriv", [C2, P, KK, 2, L], DT)
    gath = nc.dram_tensor("gath", [2, C2, P, KK, 2, L], DT,
                          addr_space="Shared")
    bar = nc.dram_tensor("bar", [2, C, 1], DT)

    with TileContext(nc) as tc:
        with (
            tc.tile_pool(name="const", bufs=1) as cpool,
            tc.tile_pool(name="zres", bufs=1) as zres,
            tc.tile_pool(name="lhs", bufs=1) as lpool,
            tc.tile_pool(name="stat", bufs=1) as spool,
            tc.tile_pool(name="work", bufs=2) as wpool,
            tc.tile_pool(name="rts", bufs=1) as rts,
            tc.tile_pool(name="esc", bufs=4) as epool,
            tc.tile_pool(name="ps", bufs=2, space="PSUM") as ps,
        ):
            pools = (cpool, zres, lpool, spool, wpool, rts, epool, ps)
            for _rep in range(reps):
                _emit_pipeline(nc, tc, z, zp, im, par, pnot, ag_in, gpriv,
                               gath, bar, nll_out, pools, phases=phases)

    nc.finalize()
    return nc


def _build_repeat(reps: int, phases: str = "ABCD"):
    return _build(reps, phases)


def get_nc():
    if "nc" not in _cached:
        _cached["nc"] = _build()
    return _cached["nc"]


def kernel(z: np.ndarray, _profile: dict | None = None) -> np.ndarray:
    assert z.shape == (N, D)
    z = np.ascontiguousarray(z, dtype=np.float32)
    imask = np.eye(P, dtype=np.float32)
    in_maps = []
    for c in range(C):
        cp = (c + 4) % C
        in_maps.append({
            "z": z[c * L:(c + 1) * L],
            "zp": z[cp * L:(cp + 1) * L],
            "im": imask,
            "par": np.array([[c & 1]], dtype=np.int32),
            "pnot": np.array([[1 - (c & 1)]], dtype=np.int32),
        })
    nc = get_nc()
    res = run_bass_kernel_spmd(nc, in_maps, core_ids=list(range(C)))
    if _profile is not None:
        _profile["exec_time_ns"] = res.exec_time_ns
        _profile["results"] = res
    # nll layout per core: [p, m] -> global row c*L + m*P + p
    total = 0.0
    for c in range(C):
        total += float(res.results[c]["nll"].sum(dtype=np.float64))
    return np.float32(total / N)
